# revision 1
# baseline (speedup 1.0000x reference)
"""Fused ConvBNReLU1D (kernel_size=1) + per-tensor po2 weight/bias fake-quant
+ QuantReLU(8-bit unsigned) output fake-quant, on 8 Trainium2 NeuronCores.

Strategy
--------
- Host: quantize W/b (per-tensor po2 scales, depends only on W/b - "precomputed
  scale" option from the sharding hint).
- Device (SPMD, data-parallel over batch B=32 -> 4 batches/core):
  Phase A: pointwise GEMM y = relu(Wq @ x + bq) with float32r matmuls
           (fp32 operands truncated to FP22 in the PE; 1 cycle/row for
           free-dim >= 256, i.e. full bf16 speed with 13 mantissa bits).
           y stays resident in SBUF (128 KiB/partition); per-chunk running
           maxes tracked on the vector engine.
  - AllReduce(max) of the per-partition max vector across the 8 cores
    (the output scale s = max(y)/255 is global).
  Phase B: out = round(y/s)*s elementwise, with round-to-nearest-even done
           via the +/- 1.5*2^23 magic-constant trick (matches jnp.round),
           then DMA out.
"""

import os
import sys
from contextlib import ExitStack

import numpy as np

for _p in ("/opt/trn_rl_repo", os.path.expanduser("~/.axon_site/_ro/trn_rl_repo")):
    if os.path.isdir(_p) and _p not in sys.path:
        sys.path.insert(0, _p)

import concourse.bacc as bacc
import concourse.mybir as mybir
import concourse.tile as tile
from concourse.bass_utils import run_bass_kernel_spmd

P = 128
B, CIN, COUT, N = 32, 512, 512, 2048
NCORES = 8
BSH = B // NCORES          # batches per core
NT = 512                   # matmul free dim (= one PSUM bank of fp32)
KT = CIN // P              # 4 contraction tiles
MT = COUT // P             # 4 output-row tiles
NJ = N // NT               # 4 n-windows per batch
NCH = BSH * NJ             # 16 (batch, n-window) chunks per core
CH2 = MT * NT              # columns of y per chunk (2048)
MAGIC = 12582912.0         # 1.5 * 2^23: RNE rounding for t in [0, 2^22)
QMAX_S = 127.0
QMAX_U = 255.0

_cache = {}
LAST_RESULT = None         # BassKernelResults of the most recent run (test.py)


def _build():
    f32 = mybir.dt.float32
    f32r = mybir.dt.float32r
    Relu = mybir.ActivationFunctionType.Relu
    Copy = mybir.ActivationFunctionType.Copy
    X = mybir.AxisListType.X
    Alu = mybir.AluOpType

    nc = bacc.Bacc(
        "TRN2",
        target_bir_lowering=False,
        debug=False,
        enable_asserts=False,
        num_devices=NCORES,
    )
    xs = nc.dram_tensor("xs", [BSH, CIN, N], f32r, kind="ExternalInput")
    wT = nc.dram_tensor("wT", [CIN, COUT], f32r, kind="ExternalInput")
    bqv = nc.dram_tensor("bqv", [P, MT], f32, kind="ExternalInput")
    out = nc.dram_tensor("out", [BSH, COUT, N], f32, kind="ExternalOutput")

    with tile.TileContext(nc) as tc, ExitStack() as ctx:
        const = ctx.enter_context(tc.tile_pool(name="const", bufs=1))
        xpool = ctx.enter_context(tc.tile_pool(name="xp", bufs=3))
        ypool = ctx.enter_context(tc.tile_pool(name="yp", bufs=1))
        pspool = ctx.enter_context(tc.tile_pool(name="ps", bufs=7, space="PSUM"))
        psb = ctx.enter_context(tc.tile_pool(name="psb", bufs=1, space="PSUM"))
        tpool = ctx.enter_context(tc.tile_pool(name="tp", bufs=3))
        dram = ctx.enter_context(tc.tile_pool(name="dram", bufs=1, space="DRAM"))

        def load_x_chunk(c):
            bb, j = divmod(c, NJ)
            xt = xpool.tile([P, KT * NT], f32r)
            # per-k-slice DMAs: matmul k waits only on its own 256 KiB slice,
            # so the PE never stalls (and never drops out of warm p-state)
            # at a chunk boundary
            for k in range(KT):
                nc.sync.dma_start(
                    out=xt[:, k * NT:(k + 1) * NT],
                    in_=xs[bb, k * P:(k + 1) * P, j * NT:(j + 1) * NT],
                )
            return xt

        # prefetch the first x chunk before the (larger) weight load so the
        # first matmul's inputs land as early as possible
        xtiles = {0: load_x_chunk(0)}

        # Weights: lhsT tile (k, m) = Wq.T[k*128:(k+1)*128, m*128:(m+1)*128],
        # packed at column (k*MT+m)*P, all in one 1 MiB DMA
        wq = const.tile([P, KT * MT * P], f32r)
        nc.sync.dma_start(
            out=wq[:, :].rearrange("p (k m q) -> p k m q", k=KT, m=MT),
            in_=wT[:, :].rearrange("(k p) (m q) -> p k m q", p=P, q=P),
        )
        bias = const.tile([P, MT], f32)
        nc.sync.dma_start(out=bias[:], in_=bqv[:, :])

        ybig = ypool.tile([P, NCH * CH2], f32)
        maxb = const.tile([P, NCH * MT], f32)

        # ---- Phase A: y = relu(Wq @ x + bq), track per-column-block maxes
        for c in range(NCH):
            xt = xtiles.pop(c) if c in xtiles else load_x_chunk(c)
            for m in range(MT):
                ps = pspool.tile([P, NT], f32)
                for k in range(KT):
                    nc.tensor.matmul(
                        ps[:],
                        wq[:, (k * MT + m) * P:(k * MT + m + 1) * P],
                        xt[:, k * NT:(k + 1) * NT],
                        start=(k == 0),
                        stop=(k == KT - 1),
                    )
                col = (c * MT + m) * NT
                nc.scalar.activation(
                    ybig[:, col:col + NT], ps[:], Relu, bias=bias[:, m:m + 1]
                )
                nc.vector.reduce_max(
                    maxb[:, c * MT + m:c * MT + m + 1],
                    ybig[:, col:col + NT],
                    axis=X,
                )

        # ---- Global max across cores (scale is global)
        mloc = const.tile([P, 1], f32)
        nc.vector.reduce_max(mloc[:], maxb[:], axis=X)
        cc_in = dram.tile([1, P], f32)
        cc_out = dram.tile([1, P], f32)
        nc.sync.dma_start(out=cc_in[:].rearrange("a b -> b a"), in_=mloc[:])
        nc.gpsimd.collective_compute(
            "AllReduce",
            Alu.max,
            replica_groups=[list(range(NCORES))],
            ins=[cc_in.opt()],
            outs=[cc_out.opt()],
        )
        grow = const.tile([1, P], f32)
        nc.sync.dma_start(out=grow[:], in_=cc_out[:])

        # sc columns: 0=gmax, 1=s, 2=inv0, 3=s*inv0, 4=2-s*inv0, 5=inv, 6=s
        sc = const.tile([1, 8], f32)
        nc.vector.reduce_max(sc[0:1, 0:1], grow[:], axis=X)
        nc.scalar.mul(sc[0:1, 1:2], sc[0:1, 0:1], 1.0 / QMAX_U)
        nc.vector.reciprocal(sc[0:1, 2:3], sc[0:1, 1:2])
        nc.vector.tensor_mul(sc[0:1, 3:4], sc[0:1, 1:2], sc[0:1, 2:3])
        nc.vector.tensor_scalar(
            out=sc[0:1, 4:5], in0=sc[0:1, 3:4],
            scalar1=-1.0, scalar2=2.0, op0=Alu.mult, op1=Alu.add,
        )
        nc.vector.tensor_mul(sc[0:1, 5:6], sc[0:1, 2:3], sc[0:1, 4:5])
        nc.vector.tensor_copy(sc[0:1, 6:7], sc[0:1, 1:2])

        # broadcast [inv, s] to all 128 partitions via a K=1 matmul with ones
        ones = const.tile([1, P], f32)
        nc.vector.memset(ones[:], 1.0)
        psc = psb.tile([P, 2], f32)
        nc.tensor.matmul(psc[:], ones[:], sc[0:1, 5:7], start=True, stop=True)
        scal = const.tile([P, 2], f32)
        nc.vector.tensor_copy(scal[:], psc[:])

        # ---- Phase B: out = round(y * inv) * s via magic-constant RNE
        for c in range(NCH):
            bb, j = divmod(c, NJ)
            t = tpool.tile([P, CH2], f32)
            nc.scalar.activation(
                t[:], ybig[:, c * CH2:(c + 1) * CH2], Copy,
                bias=MAGIC, scale=scal[:, 0:1],
            )
            nc.vector.tensor_scalar(
                out=t[:], in0=t[:],
                scalar1=-MAGIC, scalar2=scal[:, 1:2],
                op0=Alu.add, op1=Alu.mult,
            )
            # one 1 MiB DMA: [p, (m n)] -> [cout=(m p), n]
            nc.sync.dma_start(
                out=out[bb, :, j * NT:(j + 1) * NT].rearrange(
                    "(m p) n -> p m n", p=P
                ),
                in_=t[:, :].rearrange("p (m n) -> p m n", m=MT),
            )
    nc.compile()  # bacc lowering: register allocation, DCE, nop-fusion
    return nc


def _quant_po2(v, qmax):
    # mirrors reference.fake_quant_signed_po2 in float32
    v = np.asarray(v, np.float32)
    qmax = np.float32(qmax)
    maxabs = np.max(np.abs(v)).astype(np.float32)
    ratio = np.float32(maxabs / qmax)
    s = np.exp2(np.ceil(np.log2(ratio))).astype(np.float32)
    return (np.round(np.clip(v / s, -qmax, qmax)).astype(np.float32) * s).astype(
        np.float32
    )


def kernel(x, W, b):
    global LAST_RESULT
    x = np.ascontiguousarray(np.asarray(x, np.float32))
    W = np.asarray(W, np.float32)
    b = np.asarray(b, np.float32)
    assert x.shape == (B, CIN, N) and W.shape == (COUT, CIN) and b.shape == (COUT,)

    Wq = _quant_po2(W, QMAX_S)
    bq = _quant_po2(b, QMAX_S)
    wT_h = np.ascontiguousarray(Wq.T)                      # [CIN, COUT]
    bq_h = np.ascontiguousarray(bq.reshape(MT, P).T)       # [P, MT]

    if "nc" not in _cache:
        _cache["nc"] = _build()
    nc = _cache["nc"]

    in_maps = [
        {"xs": x[c * BSH:(c + 1) * BSH], "wT": wT_h, "bqv": bq_h}
        for c in range(NCORES)
    ]
    res = run_bass_kernel_spmd(nc, in_maps, core_ids=list(range(NCORES)))
    LAST_RESULT = res
    return np.concatenate(
        [res.results[c]["out"] for c in range(NCORES)], axis=0
    ).astype(np.float32)


if __name__ == "__main__":
    rng = np.random.default_rng(0)
    x = rng.standard_normal((B, CIN, N), np.float32)
    W = (rng.standard_normal((COUT, CIN)) * 0.05).astype(np.float32)
    b = (rng.standard_normal((COUT,)) * 0.1).astype(np.float32)
    y = kernel(x=x, W=W, b=b)
    print("out", y.shape, y.dtype, float(y.min()), float(y.max()))



# revision 3
# speedup vs baseline: 1.3416x; 1.3416x over previous
"""Fused ConvBNReLU1D (kernel_size=1) + per-tensor po2 weight/bias fake-quant
+ QuantReLU(8-bit unsigned) output fake-quant, on 8 Trainium2 NeuronCores.

Strategy
--------
- Host: quantize W/b (per-tensor po2 scales, depends only on W/b - "precomputed
  scale" option from the sharding hint).
- Device (SPMD, data-parallel over batch B=32 -> 4 batches/core):
  Phase A: pointwise GEMM y = relu(Wq @ x + bq) with float32r matmuls
           (fp32 operands truncated to FP22 in the PE; 1 cycle/row for
           free-dim >= 256). Weights DMA'd in 4 k-slices so the first
           matmul starts ~1.5us in. y stays resident in SBUF; per-chunk
           running maxes tracked on the vector engine.
  - AllGather of the per-partition max vectors (cheaper than AllReduce in
    both the HW and the cost model: no reduction pass), then each core
    reduces the gathered 8x128 values to the same global max locally.
  Phase B: out = round(y*inv)*s elementwise with the +/-1.5*2^23 magic
           round-to-nearest-even trick, split across the Activation engine
           (5 chunks, both passes) and the DVE (11 chunks, both passes) so
           neither engine exceeds the output-DMA time; outputs are written
           bf16 (quantized values span 8 bits, so bf16's 8+1 mantissa bits
           keep the error ~0.2% of absmax) and widened to fp32 on host.
"""

import os
import sys
from contextlib import ExitStack

import numpy as np

for _p in ("/opt/trn_rl_repo", os.path.expanduser("~/.axon_site/_ro/trn_rl_repo")):
    if os.path.isdir(_p) and _p not in sys.path:
        sys.path.insert(0, _p)

import concourse.bacc as bacc
import concourse.mybir as mybir
import concourse.tile as tile
from concourse.bass_utils import run_bass_kernel_spmd

P = 128
B, CIN, COUT, N = 32, 512, 512, 2048
NCORES = 8
BSH = B // NCORES          # batches per core
NT = 512                   # matmul free dim (= one PSUM bank of fp32)
KT = CIN // P              # 4 contraction tiles
MT = COUT // P             # 4 output-row tiles
NJ = N // NT               # 4 n-windows per batch
NCH = BSH * NJ             # 16 (batch, n-window) chunks per core
CH2 = MT * NT              # columns of y per chunk (2048)
NACT = 5                   # phase-B chunks processed on the Activation engine
MAGIC = 12582912.0         # 1.5 * 2^23: RNE rounding for t in [0, 2^22)
QMAX_S = 127.0
QMAX_U = 255.0

_cache = {}
LAST_RESULT = None         # BassKernelResults of the most recent run (test.py)


def _build():
    f32 = mybir.dt.float32
    f32r = mybir.dt.float32r
    bf16 = mybir.dt.bfloat16
    Relu = mybir.ActivationFunctionType.Relu
    Copy = mybir.ActivationFunctionType.Copy
    X = mybir.AxisListType.X
    Alu = mybir.AluOpType

    nc = bacc.Bacc(
        "TRN2",
        target_bir_lowering=False,
        debug=False,
        enable_asserts=False,
        num_devices=NCORES,
    )
    xs = nc.dram_tensor("xs", [BSH, CIN, N], f32r, kind="ExternalInput")
    wT = nc.dram_tensor("wT", [CIN, COUT], f32r, kind="ExternalInput")
    bqv = nc.dram_tensor("bqv", [P, MT], f32, kind="ExternalInput")
    out = nc.dram_tensor("out", [BSH, COUT, N], bf16, kind="ExternalOutput")

    with tile.TileContext(nc) as tc, ExitStack() as ctx:
        const = ctx.enter_context(tc.tile_pool(name="const", bufs=1))
        xpool = ctx.enter_context(tc.tile_pool(name="xp", bufs=2))
        ypool = ctx.enter_context(tc.tile_pool(name="yp", bufs=1))
        pspool = ctx.enter_context(tc.tile_pool(name="ps", bufs=7, space="PSUM"))
        psb = ctx.enter_context(tc.tile_pool(name="psb", bufs=1, space="PSUM"))
        tpool = ctx.enter_context(tc.tile_pool(name="tp", bufs=2))
        opool = ctx.enter_context(tc.tile_pool(name="op", bufs=3))
        dram = ctx.enter_context(tc.tile_pool(name="dram", bufs=1, space="DRAM"))

        # Weights: lhsT tile (k, m) = Wq.T[k*128:(k+1)*128, m*128:(m+1)*128],
        # packed at column (k*MT+m)*P.  One DMA per k-slice (256 KiB) so the
        # first matmuls only wait on their own slice.
        wq = const.tile([P, KT * MT * P], f32r)

        def load_w_slice(k):
            nc.sync.dma_start(
                out=wq[:, k * MT * P:(k + 1) * MT * P].rearrange(
                    "p (m q) -> p m q", m=MT
                ),
                in_=wT[k * P:(k + 1) * P, :].rearrange("p (m q) -> p m q", q=P),
            )

        def load_x_chunk(c):
            bb, j = divmod(c, NJ)
            xt = xpool.tile([P, KT * NT], f32r)
            for k in range(KT):
                nc.sync.dma_start(
                    out=xt[:, k * NT:(k + 1) * NT],
                    in_=xs[bb, k * P:(k + 1) * P, j * NT:(j + 1) * NT],
                )
            return xt

        # interleave: w_k0 first (smallest wait for matmul 0), then chunk-0 x
        # slices, then the rest of the weights
        load_w_slice(0)
        xtiles = {0: load_x_chunk(0)}
        for k in range(1, KT):
            load_w_slice(k)
        bias = const.tile([P, MT], f32)
        nc.sync.dma_start(out=bias[:], in_=bqv[:, :])

        ybig = ypool.tile([P, NCH * CH2], f32)
        maxb = const.tile([P, NCH * MT], f32)
        cmax = const.tile([P, NCH], f32)

        # ---- Phase A: y = relu(Wq @ x + bq), track per-chunk maxes
        for c in range(NCH):
            xt = xtiles.pop(c) if c in xtiles else load_x_chunk(c)
            for m in range(MT):
                ps = pspool.tile([P, NT], f32)
                for k in range(KT):
                    nc.tensor.matmul(
                        ps[:],
                        wq[:, (k * MT + m) * P:(k * MT + m + 1) * P],
                        xt[:, k * NT:(k + 1) * NT],
                        start=(k == 0),
                        stop=(k == KT - 1),
                    )
                col = (c * MT + m) * NT
                nc.scalar.activation(
                    ybig[:, col:col + NT], ps[:], Relu, bias=bias[:, m:m + 1]
                )
                nc.vector.reduce_max(
                    maxb[:, c * MT + m:c * MT + m + 1],
                    ybig[:, col:col + NT],
                    axis=X,
                )
            # second-level running reduce (hidden under the PE) so the
            # end-of-phase reduce only covers 16 columns
            nc.vector.reduce_max(cmax[:, c:c + 1], maxb[:, c * MT:(c + 1) * MT], axis=X)

        # ---- Global max across cores (the output scale is global)
        mloc = const.tile([P, 1], f32)
        nc.vector.reduce_max(mloc[:], cmax[:], axis=X)
        cc_in = dram.tile([1, P], f32)
        cc_out = dram.tile([1, NCORES * P], f32)
        nc.sync.dma_start(out=cc_in[:].rearrange("a b -> b a"), in_=mloc[:])
        nc.gpsimd.collective_compute(
            "AllGather",
            Alu.bypass,
            replica_groups=[list(range(NCORES))],
            ins=[cc_in.opt()],
            outs=[cc_out.opt()],
        )
        grow = const.tile([1, NCORES * P], f32)
        nc.sync.dma_start(out=grow[:], in_=cc_out[:])

        # scale chain on one partition: sc = [inv, s, -MAGIC*s] then a K=1
        # ones-matmul broadcast to all 128 partitions.
        # inv = (255/gmax) refined with one Newton step on reciprocal.
        sc = const.tile([1, 8], f32)
        gmax = sc[0:1, 3:4]
        i0 = sc[0:1, 4:5]
        e = sc[0:1, 5:6]
        nc.vector.reduce_max(gmax, grow[:], axis=X)
        nc.vector.reciprocal(i0, gmax)
        # e = 2 - gmax*i0
        nc.vector.tensor_scalar(
            out=e, in0=gmax, scalar1=i0, scalar2=-1.0,
            op0=Alu.mult, op1=Alu.mult,
        )
        nc.vector.tensor_scalar(
            out=e, in0=e, scalar1=2.0, scalar2=1.0, op0=Alu.add, op1=Alu.mult,
        )
        # inv = i0*e*255 ; s = gmax/255 ; ms = -MAGIC*s
        nc.vector.tensor_scalar(
            out=sc[0:1, 0:1], in0=e, scalar1=i0, scalar2=QMAX_U,
            op0=Alu.mult, op1=Alu.mult,
        )
        nc.vector.tensor_scalar(
            out=sc[0:1, 1:2], in0=gmax, scalar1=1.0 / QMAX_U, scalar2=1.0,
            op0=Alu.mult, op1=Alu.mult,
        )
        nc.vector.tensor_scalar(
            out=sc[0:1, 2:3], in0=sc[0:1, 1:2], scalar1=-MAGIC, scalar2=1.0,
            op0=Alu.mult, op1=Alu.mult,
        )
        ones = const.tile([1, P], f32)
        nc.vector.memset(ones[:], 1.0)
        psc = psb.tile([P, 4], f32)
        nc.tensor.matmul(psc[:], ones[:], sc[0:1, 0:4], start=True, stop=True)
        scal = const.tile([P, 4], f32)
        nc.vector.tensor_copy(scal[:], psc[:])
        inv_ap = scal[:, 0:1]
        s_ap = scal[:, 1:2]
        ms_ap = scal[:, 2:3]

        # ---- Phase B: out = round(y*inv)*s, RNE via magic constant.
        # 5 chunks run both passes on the Activation engine, 11 both on the
        # DVE; interleaved so the (FIFO) DMA queue drains in completion order.
        def chunk_act(c):
            t = tpool.tile([P, CH2], f32)
            o = opool.tile([P, CH2], bf16)
            nc.scalar.activation(
                t[:], ybig[:, c * CH2:(c + 1) * CH2], Copy,
                bias=MAGIC, scale=inv_ap,
            )
            # out = t*s - MAGIC*s >= 0 always, so Relu == identity here (and
            # unlike Copy it accepts a per-partition bias AP)
            nc.scalar.activation(o[:], t[:], Relu, bias=ms_ap, scale=s_ap)
            return o

        def chunk_dve(c):
            t = tpool.tile([P, CH2], f32)
            o = opool.tile([P, CH2], bf16)
            nc.vector.tensor_scalar(
                out=t[:], in0=ybig[:, c * CH2:(c + 1) * CH2],
                scalar1=inv_ap, scalar2=MAGIC, op0=Alu.mult, op1=Alu.add,
            )
            nc.vector.tensor_scalar(
                out=o[:], in0=t[:],
                scalar1=-MAGIC, scalar2=s_ap, op0=Alu.add, op1=Alu.mult,
            )
            return o

        acts = list(range(NACT))
        dves = list(range(NACT, NCH))
        order = []
        while acts or dves:
            for _ in range(2):
                if dves:
                    order.append((dves.pop(0), chunk_dve))
            if acts:
                order.append((acts.pop(0), chunk_act))
        for c, fn in order:
            bb, j = divmod(c, NJ)
            o = fn(c)
            nc.sync.dma_start(
                out=out[bb, :, j * NT:(j + 1) * NT].rearrange(
                    "(m p) n -> p m n", p=P
                ),
                in_=o[:, :].rearrange("p (m n) -> p m n", m=MT),
            )
    nc.compile()  # bacc lowering: register allocation, DCE, nop-fusion
    return nc


def _quant_po2(v, qmax):
    # mirrors reference.fake_quant_signed_po2 in float32
    v = np.asarray(v, np.float32)
    qmax = np.float32(qmax)
    maxabs = np.max(np.abs(v)).astype(np.float32)
    ratio = np.float32(maxabs / qmax)
    s = np.exp2(np.ceil(np.log2(ratio))).astype(np.float32)
    return (np.round(np.clip(v / s, -qmax, qmax)).astype(np.float32) * s).astype(
        np.float32
    )


def kernel(x, W, b):
    global LAST_RESULT
    x = np.ascontiguousarray(np.asarray(x, np.float32))
    W = np.asarray(W, np.float32)
    b = np.asarray(b, np.float32)
    assert x.shape == (B, CIN, N) and W.shape == (COUT, CIN) and b.shape == (COUT,)

    Wq = _quant_po2(W, QMAX_S)
    bq = _quant_po2(b, QMAX_S)
    wT_h = np.ascontiguousarray(Wq.T)                      # [CIN, COUT]
    bq_h = np.ascontiguousarray(bq.reshape(MT, P).T)       # [P, MT]

    if "nc" not in _cache:
        _cache["nc"] = _build()
    nc = _cache["nc"]

    in_maps = [
        {"xs": x[c * BSH:(c + 1) * BSH], "wT": wT_h, "bqv": bq_h}
        for c in range(NCORES)
    ]
    res = run_bass_kernel_spmd(nc, in_maps, core_ids=list(range(NCORES)))
    LAST_RESULT = res
    return np.concatenate(
        [np.asarray(res.results[c]["out"]) for c in range(NCORES)], axis=0
    ).astype(np.float32)


if __name__ == "__main__":
    rng = np.random.default_rng(0)
    x = rng.standard_normal((B, CIN, N), np.float32)
    W = (rng.standard_normal((COUT, CIN)) * 0.05).astype(np.float32)
    b = (rng.standard_normal((COUT,)) * 0.1).astype(np.float32)
    y = kernel(x=x, W=W, b=b)
    print("out", y.shape, y.dtype, float(y.min()), float(y.max()))


# revision 9
# speedup vs baseline: 1.4041x; 1.0466x over previous
"""Fused ConvBNReLU1D (kernel_size=1) + per-tensor po2 weight/bias fake-quant
+ QuantReLU(8-bit unsigned) output fake-quant, on 8 Trainium2 NeuronCores.

Strategy
--------
- Host: quantize W/b (per-tensor po2 scales, depends only on W/b - "precomputed
  scale" option from the sharding hint).
- Device (SPMD, data-parallel over batch B=32 -> 4 batches/core):
  Phase A: pointwise GEMM y = relu(Wq @ x + bq) with float32r matmuls
           (fp32 operands truncated to FP22 in the PE; 1 cycle/row for
           free-dim >= 256). Weights DMA'd in 4 k-slices so the first
           matmul starts ~1.5us in. y stays resident in SBUF; per-chunk
           running maxes tracked on the vector engine.
  - AllGather of the per-partition max vectors (cheaper than AllReduce in
    both the HW and the cost model: no reduction pass), then each core
    reduces the gathered 8x128 values to the same global max locally.
  Phase B: out = round(y*inv)*s elementwise with the +/-1.5*2^23 magic
           round-to-nearest-even trick, split across the Activation engine
           (5 chunks, both passes) and the DVE (11 chunks, both passes) so
           neither engine exceeds the output-DMA time; outputs are written
           bf16 (quantized values span 8 bits, so bf16's 8+1 mantissa bits
           keep the error ~0.2% of absmax) and widened to fp32 on host.
"""

import os
import sys
from contextlib import ExitStack

import numpy as np

for _p in ("/opt/trn_rl_repo", os.path.expanduser("~/.axon_site/_ro/trn_rl_repo")):
    if os.path.isdir(_p) and _p not in sys.path:
        sys.path.insert(0, _p)

import concourse.bacc as bacc
import concourse.mybir as mybir
import concourse.tile as tile
from concourse.bass_utils import run_bass_kernel_spmd

P = 128
B, CIN, COUT, N = 32, 512, 512, 2048
NCORES = 8
BSH = B // NCORES          # batches per core
NT = 512                   # matmul free dim (= one PSUM bank of fp32)
KT = CIN // P              # 4 contraction tiles
MT = COUT // P             # 4 output-row tiles
NJ = N // NT               # 4 n-windows per batch
NCH = BSH * NJ             # 16 (batch, n-window) chunks per core
CH2 = MT * NT              # columns of y per chunk (2048)
NACT = 6                   # phase-B chunks processed on the Activation engine
MAGIC = 12582912.0         # 1.5 * 2^23: RNE rounding for t in [0, 2^22)
QMAX_S = 127.0
QMAX_U = 255.0

_cache = {}
LAST_RESULT = None         # BassKernelResults of the most recent run (test.py)


def _build():
    f32 = mybir.dt.float32
    f32r = mybir.dt.float32r
    bf16 = mybir.dt.bfloat16
    Relu = mybir.ActivationFunctionType.Relu
    Copy = mybir.ActivationFunctionType.Copy
    X = mybir.AxisListType.X
    Alu = mybir.AluOpType

    nc = bacc.Bacc(
        "TRN2",
        target_bir_lowering=False,
        debug=False,
        enable_asserts=False,
        num_devices=NCORES,
    )
    xs = nc.dram_tensor("xs", [BSH, CIN, N], f32r, kind="ExternalInput")
    wT = nc.dram_tensor("wT", [CIN, COUT], f32r, kind="ExternalInput")
    bqv = nc.dram_tensor("bqv", [P, MT], f32, kind="ExternalInput")
    out = nc.dram_tensor("out", [BSH, COUT, N], bf16, kind="ExternalOutput")

    with tile.TileContext(nc) as tc, ExitStack() as ctx:
        const = ctx.enter_context(tc.tile_pool(name="const", bufs=1))
        xpool = ctx.enter_context(tc.tile_pool(name="xp", bufs=2))
        ypool = ctx.enter_context(tc.tile_pool(name="yp", bufs=1))
        pspool = ctx.enter_context(tc.tile_pool(name="ps", bufs=8, space="PSUM"))
        tact = ctx.enter_context(tc.tile_pool(name="ta", bufs=2))
        tdve = ctx.enter_context(tc.tile_pool(name="td", bufs=2))
        oact = ctx.enter_context(tc.tile_pool(name="oa", bufs=2))
        odve = ctx.enter_context(tc.tile_pool(name="od", bufs=2))
        dram = ctx.enter_context(tc.tile_pool(name="dram", bufs=1, space="DRAM"))

        # Weights: lhsT tile (k, m) = Wq.T[k*128:(k+1)*128, m*128:(m+1)*128],
        # packed at column (k*MT+m)*P.  One DMA per (k, m-range) slice so the
        # first matmuls only wait on their own slice.
        wq = const.tile([P, KT * MT * P], f32r)

        def load_w_slice(k, m0, m1):
            nc.sync.dma_start(
                out=wq[:, (k * MT + m0) * P:(k * MT + m1) * P].rearrange(
                    "p (m q) -> p m q", m=m1 - m0
                ),
                in_=wT[k * P:(k + 1) * P, m0 * P:m1 * P].rearrange(
                    "p (m q) -> p m q", q=P
                ),
            )

        def load_x_chunk(c):
            bb, j = divmod(c, NJ)
            xt = xpool.tile([P, KT * NT], f32r)
            for k in range(KT):
                nc.sync.dma_start(
                    out=xt[:, k * NT:(k + 1) * NT],
                    in_=xs[bb, k * P:(k + 1) * P, j * NT:(j + 1) * NT],
                )
            return xt

        # interleave: w slice (k0, m0) first (smallest wait for matmul 0),
        # then chunk-0 x slices, then the rest of the weights
        load_w_slice(0, 0, 1)
        xtiles = {0: load_x_chunk(0)}
        load_w_slice(0, 1, MT)
        for k in range(1, KT):
            load_w_slice(k, 0, MT)
        bias = const.tile([P, MT], f32)
        nc.sync.dma_start(out=bias[:], in_=bqv[:, :])

        ybig = ypool.tile([P, NCH * CH2], f32)
        maxb = const.tile([P, NCH * MT], f32)
        cmax = const.tile([P, NCH], f32)
        run = const.tile([P, 2], f32)
        mloc = const.tile([P, 1], f32)

        # ---- Phase A: y = relu(Wq @ x + bq), track per-chunk maxes
        for c in range(NCH):
            xt = xtiles.pop(c) if c in xtiles else load_x_chunk(c)
            for m in range(MT):
                ps = pspool.tile([P, NT], f32)
                for k in range(KT):
                    nc.tensor.matmul(
                        ps[:],
                        wq[:, (k * MT + m) * P:(k * MT + m + 1) * P],
                        xt[:, k * NT:(k + 1) * NT],
                        start=(k == 0),
                        stop=(k == KT - 1),
                    )
                col = (c * MT + m) * NT
                nc.scalar.activation(
                    ybig[:, col:col + NT], ps[:], Relu, bias=bias[:, m:m + 1]
                )
                nc.vector.reduce_max(
                    maxb[:, c * MT + m:c * MT + m + 1],
                    ybig[:, col:col + NT],
                    axis=X,
                )
                if c == NCH - 1 and m == MT - 2:
                    # everything except the very last (chunk, m) tile,
                    # reduced while the last matmuls run
                    nc.vector.reduce_max(run[:, 0:1], cmax[:, 0:NCH - 1], axis=X)
                    nc.vector.reduce_max(
                        run[:, 1:2],
                        maxb[:, (NCH - 1) * MT:(NCH - 1) * MT + MT - 1],
                        axis=X,
                    )
                    nc.vector.tensor_max(run[:, 0:1], run[:, 0:1], run[:, 1:2])
            if c < NCH - 1:
                # second-level running reduce (hidden under the PE)
                nc.vector.reduce_max(
                    cmax[:, c:c + 1], maxb[:, c * MT:(c + 1) * MT], axis=X
                )

        # exposed tail: only the last m-tile's max + one combine
        nc.vector.tensor_max(mloc[:], run[:, 0:1], maxb[:, NCH * MT - 1:NCH * MT])
        cc_in = dram.tile([1, P], f32)
        cc_out = dram.tile([1, NCORES * P], f32)
        nc.sync.dma_start(out=cc_in[:].rearrange("a b -> b a"), in_=mloc[:])
        nc.gpsimd.collective_compute(
            "AllGather",
            Alu.bypass,
            replica_groups=[list(range(NCORES))],
            ins=[cc_in.opt()],
            outs=[cc_out.opt()],
        )
        # transposed gather load: partition p reads rank r's max of partition p
        # at flat index r*128+p, so every partition sees all 8 ranks' values
        # and computes the (identical) scale locally - no broadcast needed.
        grow = const.tile([P, NCORES], f32)
        nc.sync.dma_start(
            out=grow[:],
            in_=cc_out[:].rearrange("a (r p) -> p (a r)", p=P),
        )

        # per-partition scale chain: scal = [inv, s, -MAGIC*s],
        # inv = 255/gmax refined with one Newton step on reciprocal.
        scal = const.tile([P, 4], f32)
        gmax = const.tile([P, 1], f32)
        i0 = const.tile([P, 1], f32)
        e = const.tile([P, 1], f32)
        nc.vector.reduce_max(gmax[:], grow[:], axis=X)
        nc.vector.reciprocal(i0[:], gmax[:])
        # e = 2 - gmax*i0
        nc.vector.tensor_scalar(
            out=e[:], in0=gmax[:], scalar1=i0[:, 0:1], scalar2=-1.0,
            op0=Alu.mult, op1=Alu.mult,
        )
        nc.vector.tensor_scalar(
            out=e[:], in0=e[:], scalar1=2.0, scalar2=1.0, op0=Alu.add, op1=Alu.mult,
        )
        # inv = i0*e*255 ; s = gmax/255 ; ms = -MAGIC*s
        nc.vector.tensor_scalar(
            out=scal[:, 0:1], in0=e[:], scalar1=i0[:, 0:1], scalar2=QMAX_U,
            op0=Alu.mult, op1=Alu.mult,
        )
        nc.vector.tensor_scalar(
            out=scal[:, 1:2], in0=gmax[:], scalar1=1.0 / QMAX_U, scalar2=1.0,
            op0=Alu.mult, op1=Alu.mult,
        )
        nc.vector.tensor_scalar(
            out=scal[:, 2:3], in0=scal[:, 1:2], scalar1=-MAGIC, scalar2=1.0,
            op0=Alu.mult, op1=Alu.mult,
        )
        inv_ap = scal[:, 0:1]
        s_ap = scal[:, 1:2]
        ms_ap = scal[:, 2:3]

        # ---- Phase B: out = round(y*inv)*s, RNE via magic constant.
        # 6 chunks run both passes on the Activation engine (~3.8us each), 10
        # both on the DVE (~2.25us each); issued in expected-completion order
        # so the (FIFO) output-DMA queue never head-of-line blocks.
        def chunk_act(c):
            t = tact.tile([P, CH2], f32)
            o = oact.tile([P, CH2], bf16)
            nc.scalar.activation(
                t[:], ybig[:, c * CH2:(c + 1) * CH2], Copy,
                bias=MAGIC, scale=inv_ap,
            )
            # out = t*s - MAGIC*s >= 0 always, so Relu == identity here (and
            # unlike Copy it accepts a per-partition bias AP)
            nc.scalar.activation(o[:], t[:], Relu, bias=ms_ap, scale=s_ap)
            return o

        def chunk_dve(c):
            t = tdve.tile([P, CH2], f32)
            o = odve.tile([P, CH2], bf16)
            nc.vector.tensor_scalar(
                out=t[:], in0=ybig[:, c * CH2:(c + 1) * CH2],
                scalar1=inv_ap, scalar2=MAGIC, op0=Alu.mult, op1=Alu.add,
            )
            nc.vector.tensor_scalar(
                out=o[:], in0=t[:],
                scalar1=-MAGIC, scalar2=s_ap, op0=Alu.add, op1=Alu.mult,
            )
            return o

        # expected per-chunk engine times (ns): DVE 2254, Act 3784
        acts = [(3784.0 * (i + 1), c, chunk_act) for i, c in enumerate(range(NACT))]
        dves = [(2254.0 * (i + 1), c, chunk_dve) for i, c in enumerate(range(NACT, NCH))]
        order = sorted(acts + dves, key=lambda t: t[0])
        for _, c, fn in order:
            bb, j = divmod(c, NJ)
            o = fn(c)
            nc.sync.dma_start(
                out=out[bb, :, j * NT:(j + 1) * NT].rearrange(
                    "(m p) n -> p m n", p=P
                ),
                in_=o[:, :].rearrange("p (m n) -> p m n", m=MT),
            )
    nc.compile()  # bacc lowering: register allocation, DCE, nop-fusion
    return nc


def _quant_po2(v, qmax):
    # mirrors reference.fake_quant_signed_po2 in float32
    v = np.asarray(v, np.float32)
    qmax = np.float32(qmax)
    maxabs = np.max(np.abs(v)).astype(np.float32)
    ratio = np.float32(maxabs / qmax)
    s = np.exp2(np.ceil(np.log2(ratio))).astype(np.float32)
    return (np.round(np.clip(v / s, -qmax, qmax)).astype(np.float32) * s).astype(
        np.float32
    )


def kernel(x, W, b):
    global LAST_RESULT
    x = np.ascontiguousarray(np.asarray(x, np.float32))
    W = np.asarray(W, np.float32)
    b = np.asarray(b, np.float32)
    assert x.shape == (B, CIN, N) and W.shape == (COUT, CIN) and b.shape == (COUT,)

    Wq = _quant_po2(W, QMAX_S)
    bq = _quant_po2(b, QMAX_S)
    wT_h = np.ascontiguousarray(Wq.T)                      # [CIN, COUT]
    bq_h = np.ascontiguousarray(bq.reshape(MT, P).T)       # [P, MT]

    if "nc" not in _cache:
        _cache["nc"] = _build()
    nc = _cache["nc"]

    in_maps = [
        {"xs": x[c * BSH:(c + 1) * BSH], "wT": wT_h, "bqv": bq_h}
        for c in range(NCORES)
    ]
    res = run_bass_kernel_spmd(nc, in_maps, core_ids=list(range(NCORES)))
    LAST_RESULT = res
    return np.concatenate(
        [np.asarray(res.results[c]["out"]) for c in range(NCORES)], axis=0
    ).astype(np.float32)


if __name__ == "__main__":
    rng = np.random.default_rng(0)
    x = rng.standard_normal((B, CIN, N), np.float32)
    W = (rng.standard_normal((COUT, CIN)) * 0.05).astype(np.float32)
    b = (rng.standard_normal((COUT,)) * 0.1).astype(np.float32)
    y = kernel(x=x, W=W, b=b)
    print("out", y.shape, y.dtype, float(y.min()), float(y.max()))
